# revision 1
# baseline (speedup 1.0000x reference)
"""ConvLSTM (2-layer, HID=64, 64x64, T=16, B=16) Trainium2 Bass kernel.

Sharding: data-parallel over batch B=16 -> 2 per NeuronCore across 8 cores;
weights/biases replicated; the sequential T-loop runs locally per core.

Per core the 3x3 convs are computed as 9 shift-offset matmuls (K=Cin,
M=128 out-channels per PSUM tile, N=512 spatial) accumulating in PSUM, in
float32r (full PE rate, ~1e-4 end-to-end error). States live in SBUF for
the whole kernel:
  inp0 [65, 2, 66, 66]  f32r : p0 = x(t) padded, p1-64 = h0(t-1)  (cell0 rhs, K=65)
  inp1 [128, 2, 66, 66] f32r : p0-63 = h0(t), p64-127 = h1        (cell1 rhs, K=128)
  c0t/c1t [128, 2, 64, 64] f32 : [64:128] = c  (base-64 alignment with f-gate)
Gates: PSUM -> ScalarE sigmoid/tanh (conv bias fused via ACT bias) -> VectorE
state update; partition-base offsets align gate/state lanes.

Within a cell, all conv matmuls are emitted before any state write so
Tile's program-order dependency tracking sees the in-place h updates
correctly; single-row seam overlaps (which Tile's subtile tracker misses)
get explicit dependency edges.
"""
import numpy as np
import concourse.tile as tile
from concourse import mybir, bacc
from concourse.bass import _add_dep_helper
from concourse.bass_utils import run_bass_kernel_spmd

F32 = mybir.dt.float32
F32R = mybir.dt.float32r
SIG = mybir.ActivationFunctionType.Sigmoid
TANH = mybir.ActivationFunctionType.Tanh
RELU = mybir.ActivationFunctionType.Relu

N_CORES = 8
B_LOC = 2
H = W = 64
HP = WP = 66
EG_ROWS = 16
CH_ROWS = 8
N_EG = H // EG_ROWS
N_MM = CH_ROWS * W  # 512


def _build(T=16):
    nc = bacc.Bacc("TRN2", target_bir_lowering=False, debug=False, num_devices=N_CORES)

    x_d = nc.dram_tensor("xp", [T, 1, B_LOC, HP, WP], F32R, kind="ExternalInput").ap()
    w0_d = nc.dram_tensor("w0t", [65, 2, 9, 128], F32R, kind="ExternalInput").ap()
    w1_d = nc.dram_tensor("w1t", [128, 2, 9, 128], F32R, kind="ExternalInput").ap()
    b_d = nc.dram_tensor("bt", [128, 4], F32, kind="ExternalInput").ap()
    wh_d = nc.dram_tensor("wht", [128, 1], F32R, kind="ExternalInput").ap()
    bh_d = nc.dram_tensor("bht", [1, 1], F32, kind="ExternalInput").ap()
    y_d = nc.dram_tensor("y", [B_LOC, H * W], F32, kind="ExternalOutput").ap()

    with tile.TileContext(nc) as tc:
        with tc.tile_pool(name="state", bufs=1) as state, \
                tc.tile_pool(name="work", bufs=2) as work, \
                tc.tile_pool(name="psp", bufs=2, space="PSUM") as psp:
            inp0 = state.tile([65, B_LOC, HP, WP], F32R)
            inp1 = state.tile([128, B_LOC, HP, WP], F32R)
            c0t = state.tile([128, B_LOC, H, W], F32)
            c1t = state.tile([128, B_LOC, H, W], F32)
            w0t = state.tile([65, 2, 9, 128], F32R)
            w1t = state.tile([128, 2, 9, 128], F32R)
            b_sb = state.tile([128, 4], F32)
            whT = state.tile([128, 1], F32R)
            bh_sb = state.tile([1, 1], F32)

            nc.sync.dma_start(out=w0t, in_=w0_d)
            nc.sync.dma_start(out=w1t, in_=w1_d)
            nc.sync.dma_start(out=b_sb, in_=b_d)
            nc.sync.dma_start(out=whT, in_=wh_d)
            nc.sync.dma_start(out=bh_sb, in_=bh_d)

            nc.vector.memset(inp0.bitcast(mybir.dt.uint32), 0)
            nc.vector.memset(inp1.bitcast(mybir.dt.uint32), 0)
            nc.vector.memset(c0t[64:128], 0.0)
            nc.vector.memset(c1t[64:128], 0.0)

            nc.sync.dma_start(out=inp0[0:1], in_=x_d[0])

            h0w = {}
            h1w = {}
            mm_info = {}

            def do_cell(cell, t):
                if cell == 0:
                    rhs_t, K, wt, ct = inp0, 65, w0t, c0t
                    h_dst, hw_d = inp1[0:64], h0w
                else:
                    rhs_t, K, wt, ct = inp1, 128, w1t, c1t
                    h_dst, hw_d = inp1[64:128], h1w
                bcol = 2 * cell
                psums = {}
                # conv phase: all matmuls before any state write
                for b in range(B_LOC):
                    for eg in range(N_EG):
                        if cell == 1:
                            info = mm_info.setdefault((t, b, eg), dict(lasts=[], dn=[], up=[]))
                        p_if = psp.tile([128, 2, N_MM], F32, tag="pif", name=f"pif_{t}_{cell}_{b}_{eg}")
                        p_og = psp.tile([128, 2, N_MM], F32, tag="pog", name=f"pog_{t}_{cell}_{b}_{eg}")
                        psums[(b, eg)] = (p_if, p_og)
                        for half in range(2):
                            r0 = eg * EG_ROWS + half * CH_ROWS
                            for off in range(9):
                                dy, dx = off // 3, off % 3
                                rhs = rhs_t[0:K, b, r0 + dy:r0 + dy + CH_ROWS, dx:dx + W]
                                st, sp = off == 0, off == 8
                                mi = nc.tensor.matmul(p_if[:, half], lhsT=wt[:, 0, off], rhs=rhs,
                                                      start=st, stop=sp)
                                mo = nc.tensor.matmul(p_og[:, half], lhsT=wt[:, 1, off], rhs=rhs,
                                                      start=st, stop=sp)
                                if cell == 1:
                                    if sp:
                                        info["lasts"] += [mi.ins, mo.ins]
                                    if half == 0 and dy == 0 and (t, b, eg - 1) in h0w:
                                        for mm in (mi, mo):
                                            _add_dep_helper(mm.ins, h0w[(t, b, eg - 1)], reason="h0 seam RAW dn")
                                    if half == 1 and dy == 2 and (t, b, eg + 1) in h0w:
                                        for mm in (mi, mo):
                                            _add_dep_helper(mm.ins, h0w[(t, b, eg + 1)], reason="h0 seam RAW up")
                # elementwise phase
                for b in range(B_LOC):
                    for eg in range(N_EG):
                        p_if, p_og = psums[(b, eg)]
                        pif_f = p_if.rearrange("p a b -> p (a b)")
                        pog_f = p_og.rearrange("p a b -> p (a b)")
                        NE = 2 * N_MM
                        if_h = work.tile([128, NE], F32, tag="ifh", name=f"ifh_{t}_{cell}_{b}_{eg}")
                        g_h = work.tile([64, NE], F32, tag="gh", name=f"gh_{t}_{cell}_{b}_{eg}")
                        o_h = work.tile([64, NE], F32, tag="oh", name=f"oh_{t}_{cell}_{b}_{eg}")
                        m1 = work.tile([128, NE], F32, tag="m1", name=f"m1_{t}_{cell}_{b}_{eg}")
                        m2 = work.tile([128, NE], F32, tag="m2", name=f"m2_{t}_{cell}_{b}_{eg}")
                        t5 = work.tile([64, NE], F32, tag="t5", name=f"t5_{t}_{cell}_{b}_{eg}")
                        nc.scalar.activation(out=if_h, in_=pif_f, func=SIG,
                                             bias=b_sb[:, bcol:bcol + 1])
                        nc.scalar.activation(out=g_h, in_=pog_f[64:128], func=TANH,
                                             bias=b_sb[64:128, bcol + 1:bcol + 2])
                        nc.scalar.activation(out=o_h, in_=pog_f[0:64], func=SIG,
                                             bias=b_sb[0:64, bcol + 1:bcol + 2])
                        cseg = ct[64:128, b].rearrange("p a b -> p (a b)")[:, eg * EG_ROWS * W:(eg + 1) * EG_ROWS * W]
                        nc.vector.tensor_mul(m1[64:128], if_h[64:128], cseg)
                        nc.vector.tensor_mul(m2[64:128], if_h[0:64], g_h)
                        nc.vector.tensor_add(cseg, m1[64:128], m2[64:128])
                        nc.scalar.activation(out=t5, in_=cseg, func=TANH)
                        hseg = h_dst[:, b, 1 + eg * EG_ROWS:1 + (eg + 1) * EG_ROWS, 1:1 + W]
                        hw = nc.vector.tensor_mul(hseg, o_h, t5)
                        hw_d[(t, b, eg)] = hw.ins
                        if cell == 1:
                            for dg in (-1, 1):
                                if (t, b, eg + dg) in mm_info:
                                    for mm in mm_info[(t, b, eg + dg)]["lasts"]:
                                        _add_dep_helper(hw.ins, mm, reason="h1 seam WAR")

            for t in range(T):
                do_cell(0, t)
                nc.sync.dma_start(out=inp0[1:65], in_=inp1[0:64])
                if t + 1 < T:
                    nc.sync.dma_start(out=inp0[0:1], in_=x_d[t + 1])
                do_cell(1, t)

            for b in range(B_LOC):
                for ch in range(H // CH_ROWS):
                    p_h = psp.tile([1, N_MM], F32, tag="pif", name=f"ph_{b}_{ch}")
                    rhs = inp1[64:128, b, 1 + ch * CH_ROWS:1 + (ch + 1) * CH_ROWS, 1:1 + W]
                    mh = nc.tensor.matmul(p_h, lhsT=whT[64:128], rhs=rhs, start=True, stop=True)
                    if (T - 1, b, ch // 2) in h1w:
                        _add_dep_helper(mh.ins, h1w[(T - 1, b, ch // 2)], reason="head RAW")
                    h_out = work.tile([1, N_MM], F32, tag="ho", name=f"ho_{b}_{ch}")
                    nc.scalar.activation(out=h_out, in_=p_h, func=RELU, bias=bh_sb[0:1, 0:1])
                    nc.sync.dma_start(out=y_d[b:b + 1, ch * N_MM:(ch + 1) * N_MM], in_=h_out)

    nc.compile()
    return nc


def _prep_inputs(x, w0, b0, w1, b1, wh, bh):
    x = np.asarray(x, np.float32)
    B, T = x.shape[0], x.shape[1]
    bl = B // N_CORES

    def wprep(w, K):
        wt = np.asarray(w, np.float32).reshape(2, 128, K, 3, 3)
        wt = np.transpose(wt, (2, 0, 3, 4, 1))
        return np.ascontiguousarray(wt.reshape(K, 2, 9, 128))

    w0t = wprep(w0, 65)
    w1t = wprep(w1, 128)
    b0 = np.asarray(b0, np.float32)
    b1 = np.asarray(b1, np.float32)
    bt = np.stack([b0[0:128], b0[128:256], b1[0:128], b1[128:256]], axis=1).astype(np.float32)
    wht = np.zeros((128, 1), np.float32)
    wht[64:128, 0] = np.asarray(wh, np.float32).reshape(64)
    bht = np.array([[float(np.asarray(bh).reshape(-1)[0])]], np.float32)

    xp_all = np.zeros((B, T, 1, HP, WP), np.float32)
    xp_all[:, :, 0, 1:1 + H, 1:1 + W] = x[:, :, 0]

    in_maps = []
    for c in range(N_CORES):
        xp = np.ascontiguousarray(xp_all[c * bl:(c + 1) * bl].transpose(1, 2, 0, 3, 4))
        in_maps.append({"xp": xp, "w0t": w0t, "w1t": w1t, "bt": bt,
                        "wht": wht, "bht": bht})
    return in_maps


_NC_CACHE = {}


def kernel(x, w0, b0, w1, b1, wh, bh):
    x = np.asarray(x)
    B, T = x.shape[0], x.shape[1]
    if T not in _NC_CACHE:
        _NC_CACHE[T] = _build(T=T)
    nc = _NC_CACHE[T]
    in_maps = _prep_inputs(x, w0, b0, w1, b1, wh, bh)
    res = run_bass_kernel_spmd(nc, in_maps, core_ids=list(range(N_CORES)))
    bl = B // N_CORES
    out = np.zeros((B, 1, H, W), np.float32)
    for c, r in enumerate(res.results):
        out[c * bl:(c + 1) * bl, 0] = r["y"].reshape(bl, H, W)
    return out



# revision 2
# speedup vs baseline: 1.0074x; 1.0074x over previous
"""ConvLSTM (2-layer, HID=64, 64x64, T=16, B=16) Trainium2 Bass kernel, v2.

Sharding: data-parallel over batch B=16 -> 2 per NeuronCore across 8 cores;
weights/biases replicated; the sequential T-loop runs locally per core.

v2 over baseline:
- cell0 conv restructured: the x-contribution is a single K=9 matmul per
  (tgt,half) against a host-built 9-tap im2col of x; the recurrent part
  runs K=128 paired-tap matmuls against [h0 ; h0-colshift1] (two taps per
  matmul) plus 3 K=64 leftover-tap matmuls -> 7 matmuls per gate tile
  instead of 9 half-empty K=65 ones (~11% less TensorE).
- elementwise phase: gates packed two-per-PSUM-target so every sigmoid is
  a 128-partition ACT; tanh(g) = 2*sigmoid(2g)-1 with the 2x folded into
  the g-weights; state-update DVE ops pair the 2 local batches into
  [64, 2048] ops. All multi-input ops keep equal partition bases (HW rule);
  results are placed via output partition shifts.

States in SBUF for the whole kernel:
  A0 [128, 2, 66, 66] f32r : [h0 ; h0 shifted left 1 col] (cell0 rhs)
  A1 [128, 2, 66, 66] f32r : [h0 ; h1]                    (cell1 rhs)
  ct01 [128, 2, 64, 64] f32 : c0 at parts 0:64, c1 at parts 64:128
A0 is refreshed from A1[0:64] by two whole-tile DMAs per step. Gate target
layouts: cell0 A=[f;i], cell1 A=[i;f] (so m1's operands share base), both
cells B=[o;2g]. Seam row overlaps Tile's subtile tracker misses get
explicit dependency edges (pattern inherited from the v1 kernel).
"""
import numpy as np
import concourse.tile as tile
from concourse import mybir, bacc
from concourse.bass import _add_dep_helper
from concourse.bass_utils import run_bass_kernel_spmd

F32 = mybir.dt.float32
F32R = mybir.dt.float32r
SIG = mybir.ActivationFunctionType.Sigmoid
TANH = mybir.ActivationFunctionType.Tanh
COPY = mybir.ActivationFunctionType.Copy
RELU = mybir.ActivationFunctionType.Relu

N_CORES = 8
B_LOC = 2
H = W = 64
HP = WP = 66
EG_ROWS = 16
CH_ROWS = 8
N_EG = H // EG_ROWS  # 4
N_MM = CH_ROWS * W  # 512


def _build(T=16):
    nc = bacc.Bacc("TRN2", target_bir_lowering=False, debug=False, num_devices=N_CORES)

    x3_d = nc.dram_tensor("x3", [T, 3, B_LOC, HP, WP], F32R,
                          kind="ExternalInput").ap()
    w0p_d = nc.dram_tensor("w0p", [128, 2, 3, 128], F32R, kind="ExternalInput").ap()
    w0s_d = nc.dram_tensor("w0s", [128, 2, 3, 128], F32R, kind="ExternalInput").ap()
    w1_d = nc.dram_tensor("w1t", [128, 2, 9, 128], F32R, kind="ExternalInput").ap()
    b_d = nc.dram_tensor("bt", [128, 4], F32, kind="ExternalInput").ap()
    wh_d = nc.dram_tensor("wht", [128, 1], F32R, kind="ExternalInput").ap()
    bh_d = nc.dram_tensor("bht", [1, 1], F32, kind="ExternalInput").ap()
    y_d = nc.dram_tensor("y", [B_LOC, H * W], F32, kind="ExternalOutput").ap()

    with tile.TileContext(nc) as tc:
        with tc.tile_pool(name="state", bufs=1) as state, \
                tc.tile_pool(name="work", bufs=2) as work, \
                tc.tile_pool(name="psp", bufs=2, space="PSUM") as psp:
            A0 = state.tile([128, B_LOC, HP, WP], F32R)
            # A0b: x3 planes at parts 0:3, h0-colshift copy at parts 64:128;
            # serves the fused x+dy2-tap matmuls
            A0b = state.tile([128, B_LOC, HP, WP], F32R)
            A1 = state.tile([128, B_LOC, HP, WP], F32R)
            ct01 = state.tile([128, B_LOC, H, W], F32)
            w0p = state.tile([128, 2, 3, 128], F32R)
            w0s = state.tile([128, 2, 3, 128], F32R)
            w1t = state.tile([128, 2, 9, 128], F32R)
            b_sb = state.tile([128, 4], F32)
            whT = state.tile([128, 1], F32R)
            bh_sb = state.tile([1, 1], F32)

            nc.sync.dma_start(out=w0p, in_=w0p_d)
            nc.sync.dma_start(out=w0s, in_=w0s_d)
            nc.sync.dma_start(out=w1t, in_=w1_d)
            nc.sync.dma_start(out=b_sb, in_=b_d)
            nc.sync.dma_start(out=whT, in_=wh_d)
            nc.sync.dma_start(out=bh_sb, in_=bh_d)

            nc.vector.memset(A0.bitcast(mybir.dt.uint32), 0)
            nc.vector.memset(A0b.bitcast(mybir.dt.uint32), 0)
            nc.vector.memset(A1.bitcast(mybir.dt.uint32), 0)
            nc.vector.memset(ct01, 0.0)

            h0w = {}   # (t, eg) -> h0-write instruction (covers both b)
            h1w = {}   # (t, eg) -> h1-write instruction
            mm1_last = {}  # (t, b, eg) -> last matmuls of cell1 group

            # x3(t=0) must land before the first conv
            nc.sync.dma_start(out=A0b[0:3], in_=x3_d[0])

            def cell0_convs(t):
                psums = {}
                # eg-outer to match the ew phase's consumption order (the
                # ScalarE FIFO + 2-deep psum pool deadlock otherwise)
                for eg in range(N_EG):
                    for b in range(B_LOC):
                        pA = psp.tile([128, 2, N_MM], F32, tag="pA",
                                      name=f"p0A_{t}_{b}_{eg}")
                        pB = psp.tile([128, 2, N_MM], F32, tag="pB",
                                      name=f"p0B_{t}_{b}_{eg}")
                        psums[(b, eg)] = (pA, pB)
                        for tgt, p in ((0, pA), (1, pB)):
                            for dy in range(3):
                                for half in range(2):
                                    r0 = eg * EG_ROWS + half * CH_ROWS
                                    rhs = A0[0:128, b, r0 + dy:r0 + dy + CH_ROWS, 0:W]
                                    nc.tensor.matmul(p[:, half], lhsT=w0p[:, tgt, dy],
                                                     rhs=rhs, start=(dy == 0),
                                                     stop=False)
                            for dy in range(3):
                                for half in range(2):
                                    r0 = eg * EG_ROWS + half * CH_ROWS
                                    # fused tap: x3 planes on parts 0:3 (x taps
                                    # dy,0..2), zeros on 3:64, h0-colshift on
                                    # 64:128 (h tap (dy,2)); full-width K=128
                                    rhs = A0b[0:128, b, r0 + dy:r0 + dy + CH_ROWS, 1:1 + W]
                                    nc.tensor.matmul(p[:, half], lhsT=w0s[:, tgt, dy],
                                                     rhs=rhs, start=False,
                                                     stop=(dy == 2))
                return psums

            def cell1_convs(t):
                psums = {}
                for eg in range(N_EG):
                    for b in range(B_LOC):
                        pA = psp.tile([128, 2, N_MM], F32, tag="pA",
                                      name=f"p1A_{t}_{b}_{eg}")
                        pB = psp.tile([128, 2, N_MM], F32, tag="pB",
                                      name=f"p1B_{t}_{b}_{eg}")
                        psums[(b, eg)] = (pA, pB)
                        lasts = []
                        for tgt, p in ((0, pA), (1, pB)):
                            for off in range(9):
                                dy, dx = off // 3, off % 3
                                for half in range(2):
                                    r0 = eg * EG_ROWS + half * CH_ROWS
                                    rhs = A1[0:128, b, r0 + dy:r0 + dy + CH_ROWS, dx:dx + W]
                                    mm = nc.tensor.matmul(p[:, half], lhsT=w1t[:, tgt, off],
                                                          rhs=rhs, start=(off == 0),
                                                          stop=(off == 8))
                                    if off == 8:
                                        lasts.append(mm.ins)
                                    # seam RAW: boundary rows written by the
                                    # neighboring eg's h-updates of A1
                                    if half == 0 and dy == 0:
                                        for hw_d, tt in ((h0w, t), (h1w, t - 1)):
                                            if (tt, eg - 1) in hw_d:
                                                _add_dep_helper(mm.ins, hw_d[(tt, eg - 1)],
                                                                reason="seam RAW dn")
                                    if half == 1 and dy == 2:
                                        for hw_d, tt in ((h0w, t), (h1w, t - 1)):
                                            if (tt, eg + 1) in hw_d:
                                                _add_dep_helper(mm.ins, hw_d[(tt, eg + 1)],
                                                                reason="seam RAW up")
                        mm1_last[(t, b, eg)] = lasts
                return psums

            def cell_ew(cell, t, psums):
                # gate targets: cell0 A=[f;i], cell1 A=[i;f]; both B=[o;2g].
                # c-state: c0 at ct01[0:64], c1 at ct01[64:128]. All two-input
                # ops below read operands at one shared partition base; results
                # land via output partition shifts.
                bcol = 2 * cell
                clo, chi = (0, 64) if cell == 0 else (64, 128)   # c partition range
                flo, fhi = (0, 64) if cell == 0 else (64, 128)   # sig(f) in sA
                ilo, ihi = (64, 128) if cell == 0 else (0, 64)   # sig(i) in sA
                hw_d = h0w if cell == 0 else h1w
                for eg in range(N_EG):
                    seg = slice(eg * EG_ROWS, (eg + 1) * EG_ROWS)
                    sA = work.tile([128, B_LOC, 2 * N_MM], F32, tag="sA",
                                   name=f"sA_{t}_{cell}_{eg}")
                    sB = work.tile([128, B_LOC, 2 * N_MM], F32, tag="sB",
                                   name=f"sB_{t}_{cell}_{eg}")
                    M = work.tile([128, B_LOC, 2 * N_MM], F32, tag="M",
                                  name=f"M_{t}_{cell}_{eg}")
                    for b in range(B_LOC):
                        pA, pB = psums[(b, eg)]
                        nc.scalar.activation(out=sA[:, b],
                                             in_=pA.rearrange("p a b -> p (a b)"),
                                             func=SIG, bias=b_sb[:, bcol:bcol + 1])
                        nc.scalar.activation(out=sB[:, b],
                                             in_=pB.rearrange("p a b -> p (a b)"),
                                             func=SIG, bias=b_sb[:, bcol + 1:bcol + 2])
                    cseg = ct01[clo:chi, :, seg].rearrange("p a b c -> p a (b c)")
                    # tg = 2*sig(2g) - 1, placed at the i-gate's partition base
                    nc.scalar.activation(out=M[ilo:ihi], in_=sB[64:128], func=COPY,
                                         bias=-1.0, scale=2.0)
                    # m2 = sig(i) * tg -> sB at c's base (sig(2g) is dead)
                    nc.vector.tensor_mul(sB[64:128], sA[ilo:ihi], M[ilo:ihi])
                    # m1 = sig(f) * c -> M at c's... staged at base 64
                    nc.vector.tensor_mul(M[64:128], sA[flo:fhi], cseg)
                    # c' = m1 + m2 -> ct01
                    nc.vector.tensor_add(cseg, sB[64:128], M[64:128])
                    # t5 = tanh(c') -> M[0:64]
                    nc.scalar.activation(out=M[0:64], in_=cseg, func=TANH)
                    # h = sig(o) * t5 -> A1
                    hdst = A1[64 * cell:64 * cell + 64, :,
                              1 + eg * EG_ROWS:1 + (eg + 1) * EG_ROWS, 1:1 + W]
                    hw = nc.vector.tensor_mul(hdst, sB[0:64], M[0:64])
                    hw_d[(t, eg)] = hw.ins
                    if cell == 0:
                        # WAR: previous step's cell1 matmuls read seam rows of
                        # the h0 region this write replaces
                        for dg in (-1, 1):
                            for b in range(B_LOC):
                                if (t - 1, b, eg + dg) in mm1_last:
                                    for mm in mm1_last[(t - 1, b, eg + dg)]:
                                        _add_dep_helper(hw.ins, mm, reason="h0 seam WAR")
                    else:
                        for dg in (-1, 1):
                            for b in range(B_LOC):
                                if (t, b, eg + dg) in mm1_last:
                                    for mm in mm1_last[(t, b, eg + dg)]:
                                        _add_dep_helper(hw.ins, mm, reason="h1 seam WAR")

            for t in range(T):
                ps0 = cell0_convs(t)
                cell_ew(0, t, ps0)
                if t + 1 < T:
                    # refresh A0 = [h0 ; h0 colshift] and A0b's colshift copy
                    # for the next step; the col shift is a flat 1-element
                    # shift (row-wrap elements all land on zero padding),
                    # giving contiguous DMAs
                    A0f = A0.rearrange("p a b c -> p (a b c)")
                    A0bf = A0b.rearrange("p a b c -> p (a b c)")
                    A1f = A1.rearrange("p a b c -> p (a b c)")
                    nflat = B_LOC * HP * WP
                    nc.sync.dma_start(out=A0f[0:64], in_=A1f[0:64])
                    nc.sync.dma_start(out=A0f[64:128, 0:nflat - 1],
                                      in_=A1f[0:64, 1:nflat])
                    nc.sync.dma_start(out=A0bf[64:128, 0:nflat - 1],
                                      in_=A1f[0:64, 1:nflat])
                    nc.sync.dma_start(out=A0b[0:3], in_=x3_d[t + 1])
                ps1 = cell1_convs(t)
                cell_ew(1, t, ps1)

            for b in range(B_LOC):
                for ch in range(H // CH_ROWS):
                    p_h = psp.tile([1, N_MM], F32, tag="pA", name=f"ph_{b}_{ch}")
                    rhs = A1[64:128, b, 1 + ch * CH_ROWS:1 + (ch + 1) * CH_ROWS, 1:1 + W]
                    mh = nc.tensor.matmul(p_h, lhsT=whT[64:128], rhs=rhs,
                                          start=True, stop=True)
                    if (T - 1, ch // 2) in h1w:
                        _add_dep_helper(mh.ins, h1w[(T - 1, ch // 2)], reason="head RAW")
                    h_out = work.tile([1, N_MM], F32, tag="ho", name=f"ho_{b}_{ch}")
                    nc.scalar.activation(out=h_out, in_=p_h, func=RELU,
                                         bias=bh_sb[0:1, 0:1])
                    nc.sync.dma_start(out=y_d[b:b + 1, ch * N_MM:(ch + 1) * N_MM],
                                      in_=h_out)

    nc.compile()
    return nc


def _prep_inputs(x, w0, b0, w1, b1, wh, bh):
    x = np.asarray(x, np.float32)
    B, T = x.shape[0], x.shape[1]
    bl = B // N_CORES

    def gate_blocks(w, b, K, a_order):
        # w [256, K, 3, 3], b [256]; gate order i,f,o,g
        w = np.asarray(w, np.float32).reshape(4, 64, K, 3, 3)
        b = np.asarray(b, np.float32).reshape(4, 64)
        wi, wf, wo, wg = w
        bi, bf, bo, bg = b
        if a_order == "fi":
            wA = np.concatenate([wf, wi], axis=0)
            bA = np.concatenate([bf, bi])
        else:
            wA = np.concatenate([wi, wf], axis=0)
            bA = np.concatenate([bi, bf])
        wB = np.concatenate([wo, 2.0 * wg], axis=0)
        bB = np.concatenate([bo, 2.0 * bg])
        return wA, wB, bA, bB

    # cell0: input channels [x(1); h0(64)]; A = [f; i]
    w0A, w0B, b0A, b0B = gate_blocks(w0, b0, 65, "fi")
    w0p = np.zeros((128, 2, 3, 128), np.float32)
    w0s = np.zeros((128, 2, 3, 128), np.float32)
    for tgt, wt in ((0, w0A), (1, w0B)):
        wh_ = wt[:, 1:65]   # [128col, 64ch, 3, 3]
        for dy in range(3):
            w0p[0:64, tgt, dy, :] = wh_[:, :, dy, 0].T
            w0p[64:128, tgt, dy, :] = wh_[:, :, dy, 1].T
            w0s[64:128, tgt, dy, :] = wh_[:, :, dy, 2].T
            # x taps (dy, 0..2) ride the dy2 matmul on partitions 0:3
            for dxp in range(3):
                w0s[dxp, tgt, dy, :] = wt[:, 0, dy, dxp]

    # cell1: input channels [h0(64); h1(64)]; A = [i; f]
    w1A, w1B, b1A, b1B = gate_blocks(w1, b1, 128, "if")
    w1t = np.zeros((128, 2, 9, 128), np.float32)
    for tgt, wt in ((0, w1A), (1, w1B)):
        w1t[:, tgt, :, :] = np.transpose(wt.reshape(128, 128, 9), (1, 2, 0))

    bt = np.stack([b0A, b0B, b1A, b1B], axis=1).astype(np.float32)

    wht = np.zeros((128, 1), np.float32)
    wht[64:128, 0] = np.asarray(wh, np.float32).reshape(64)
    bht = np.array([[float(np.asarray(bh).reshape(-1)[0])]], np.float32)

    # x3 planes: X3[t, dxp, b, r, cc] = x_pad[b, t, r, cc-1+dxp]
    # (read at rhs col cc=c+1, row r0+dy -> x_pad[r+dy, c+dxp] = tap (dy,dxp))
    xpad = np.zeros((B, T, HP, WP), np.float32)
    xpad[:, :, 1:1 + H, 1:1 + W] = x[:, :, 0]
    xpe = np.zeros((B, T, HP, WP + 2), np.float32)
    xpe[:, :, :, 1:1 + WP] = xpad
    x3_all = np.zeros((B, T, 3, HP, WP), np.float32)
    for dxp in range(3):
        x3_all[:, :, dxp] = xpe[:, :, :, dxp:dxp + WP]

    in_maps = []
    for c in range(N_CORES):
        # -> [T, 3, b, HP, WP]
        x3 = np.ascontiguousarray(
            x3_all[c * bl:(c + 1) * bl].transpose(1, 2, 0, 3, 4))
        in_maps.append({"x3": x3, "w0p": w0p, "w0s": w0s,
                        "w1t": w1t, "bt": bt, "wht": wht, "bht": bht})
    return in_maps


_NC_CACHE = {}


def kernel(x, w0, b0, w1, b1, wh, bh):
    x = np.asarray(x)
    B, T = x.shape[0], x.shape[1]
    if T not in _NC_CACHE:
        _NC_CACHE[T] = _build(T=T)
    nc = _NC_CACHE[T]
    in_maps = _prep_inputs(x, w0, b0, w1, b1, wh, bh)
    res = run_bass_kernel_spmd(nc, in_maps, core_ids=list(range(N_CORES)))
    bl = B // N_CORES
    out = np.zeros((B, 1, H, W), np.float32)
    for c, r in enumerate(res.results):
        out[c * bl:(c + 1) * bl, 0] = r["y"].reshape(bl, H, W)
    return out


# revision 5
# speedup vs baseline: 1.1654x; 1.1569x over previous
"""ConvLSTM (2-layer, HID=64, 64x64, T=16, B=16) Trainium2 Bass kernel.

Sharding: data-parallel over batch B=16 -> 2 per NeuronCore across 8 cores;
weights/biases replicated; the sequential T-loop runs locally per core.
Matmul operands are bf16 (PSUM accumulation and the state update stay fp32);
bf16 streams at full PE rate with fast weight loads and halves SBUF, which
is what makes the 5-matmul cell0 decomposition below fit.

cell0 (65->256, 3x3) runs FIVE full-width K=128 matmuls per 128-out x
512-pos gate tile (vs 9 half-empty K=65 ones naively):
  - 3 paired-tap matmuls vs A0  = [h0 ; h0<<1col]   (taps (dy,0)+(dy,1))
  - 1 paired-tap matmul  vs A0c = [h0<<2col ; h0<<2col,+1row]  ((0,2)+(1,2))
  - 1 fused matmul       vs A0d = [x im2col 9 planes ; 0 ; h0<<2col]
    covering h tap (2,2) plus ALL NINE x taps on partitions 0:9
At t=0 all h taps vanish (h0=0): one matmul per gate tile.
cell1 (128->256, 3x3) is 9 K=128 shifted-window matmuls per gate tile
(PE-optimal). Head: K=64 1x1 matmuls + fused bias-relu ACT.

The h0 shift copies are flat 1/2/68-element-shifted contiguous DMAs from
A1[0:64] each step; row-wrap elements land on padding or unread columns.

Elementwise: gates packed [f;i] (cell0) / [i;f] (cell1) and [o;2g]; every
sigmoid is a 128-partition ACT; tanh(g) = 2*sigmoid(2g)-1 with the 2x
folded into the g-weights; DVE ops pair the 2 local batches ([64,2048]).
All two-input engine ops keep equal partition bases (HW rule); placement
via output partition shifts. Seam row overlaps Tile's subtile tracker
misses get explicit dependency edges.

States in SBUF: A0/A0c/A0d/A1 [128, 2, 66, 66] bf16,
ct01 [128, 2, 64, 64] f32 (c0 at parts 0:64, c1 at parts 64:128).
"""
import numpy as np
import ml_dtypes
import concourse.tile as tile
from concourse import mybir, bacc
from concourse.bass import _add_dep_helper
from concourse.bass_utils import run_bass_kernel_spmd

F32 = mybir.dt.float32
BF16 = mybir.dt.bfloat16
SIG = mybir.ActivationFunctionType.Sigmoid
TANH = mybir.ActivationFunctionType.Tanh
COPY = mybir.ActivationFunctionType.Copy
RELU = mybir.ActivationFunctionType.Relu

N_CORES = 8
B_LOC = 2
H = W = 64
HP = WP = 66
EG_ROWS = 16
CH_ROWS = 8
N_EG = H // EG_ROWS  # 4
N_MM = CH_ROWS * W  # 512


def _build(T=16):
    nc = bacc.Bacc("TRN2", target_bir_lowering=False, debug=False, num_devices=N_CORES)

    x9_d = nc.dram_tensor("x9", [T, 9, B_LOC, HP, WP], BF16,
                          kind="ExternalInput").ap()
    w0p_d = nc.dram_tensor("w0p", [128, 2, 3, 128], BF16, kind="ExternalInput").ap()
    w0c_d = nc.dram_tensor("w0c", [128, 2, 128], BF16, kind="ExternalInput").ap()
    w0d_d = nc.dram_tensor("w0d", [128, 2, 128], BF16, kind="ExternalInput").ap()
    w1_d = nc.dram_tensor("w1t", [128, 2, 9, 128], BF16, kind="ExternalInput").ap()
    b_d = nc.dram_tensor("bt", [128, 4], F32, kind="ExternalInput").ap()
    wh_d = nc.dram_tensor("wht", [128, 1], BF16, kind="ExternalInput").ap()
    bh_d = nc.dram_tensor("bht", [1, 1], F32, kind="ExternalInput").ap()
    y_d = nc.dram_tensor("y", [B_LOC, H * W], F32, kind="ExternalOutput").ap()

    with tile.TileContext(nc) as tc:
        with tc.tile_pool(name="state", bufs=1) as state, \
                tc.tile_pool(name="work", bufs=2) as work, \
                tc.tile_pool(name="psp", bufs=2, space="PSUM") as psp:
            A0 = state.tile([128, B_LOC, HP, WP], BF16)
            A0c = state.tile([128, B_LOC, HP, WP], BF16)
            A0d = state.tile([128, B_LOC, HP, WP], BF16)
            A1 = state.tile([128, B_LOC, HP, WP], BF16)
            ct01 = state.tile([128, B_LOC, H, W], F32)
            w0p = state.tile([128, 2, 3, 128], BF16)
            w0c = state.tile([128, 2, 128], BF16)
            w0d = state.tile([128, 2, 128], BF16)
            w1t = state.tile([128, 2, 9, 128], BF16)
            b_sb = state.tile([128, 4], F32)
            whT = state.tile([128, 1], BF16)
            bh_sb = state.tile([1, 1], F32)

            # t=0-critical first: w0d + A0d (its memset + the x im2col DMA)
            nc.sync.dma_start(out=w0d, in_=w0d_d)
            nc.vector.memset(A0d.bitcast(mybir.dt.uint16), 0)
            nc.sync.dma_start(out=A0d[0:9], in_=x9_d[0])
            nc.sync.dma_start(out=w0p, in_=w0p_d)
            nc.sync.dma_start(out=w0c, in_=w0c_d)
            nc.sync.dma_start(out=w1t, in_=w1_d)
            nc.sync.dma_start(out=b_sb, in_=b_d)
            nc.sync.dma_start(out=whT, in_=wh_d)
            nc.sync.dma_start(out=bh_sb, in_=bh_d)

            nc.vector.memset(A1.bitcast(mybir.dt.uint16), 0)
            nc.vector.memset(ct01, 0.0)
            nc.vector.memset(A0.bitcast(mybir.dt.uint16), 0)
            nc.vector.memset(A0c.bitcast(mybir.dt.uint16), 0)

            h0w = {}   # (t, eg) -> h0-write instruction (covers both b)
            h1w = {}   # (t, eg) -> h1-write instruction
            mm1_last = {}  # (t, b, eg) -> last matmuls of cell1 group

            def cell0_convs(t):
                psums = {}
                # eg-outer to match the ew phase's consumption order (the
                # ScalarE FIFO + 2-deep psum pool deadlock otherwise)
                for eg in range(N_EG):
                    for b in range(B_LOC):
                        pA = psp.tile([128, 2, N_MM], F32, tag="pA",
                                      name=f"p0A_{t}_{b}_{eg}")
                        pB = psp.tile([128, 2, N_MM], F32, tag="pB",
                                      name=f"p0B_{t}_{b}_{eg}")
                        psums[(b, eg)] = (pA, pB)
                        for tgt, p in ((0, pA), (1, pB)):
                            # t=0: h0 == 0, only the x-carrying matmul matters
                            for dy in (range(3) if t > 0 else ()):
                                for half in range(2):
                                    r0 = eg * EG_ROWS + half * CH_ROWS
                                    rhs = A0[0:128, b, r0 + dy:r0 + dy + CH_ROWS, 0:W]
                                    nc.tensor.matmul(p[:, half], lhsT=w0p[:, tgt, dy],
                                                     rhs=rhs, start=(dy == 0),
                                                     stop=False)
                            if t > 0:
                                for half in range(2):
                                    r0 = eg * EG_ROWS + half * CH_ROWS
                                    rhs = A0c[0:128, b, r0:r0 + CH_ROWS, 0:W]
                                    nc.tensor.matmul(p[:, half], lhsT=w0c[:, tgt],
                                                     rhs=rhs, start=False, stop=False)
                            for half in range(2):
                                r0 = eg * EG_ROWS + half * CH_ROWS
                                rhs = A0d[0:128, b, r0 + 2:r0 + 2 + CH_ROWS, 0:W]
                                nc.tensor.matmul(p[:, half], lhsT=w0d[:, tgt],
                                                 rhs=rhs, start=(t == 0), stop=True)
                return psums

            def cell1_convs(t):
                psums = {}
                for eg in range(N_EG):
                    for b in range(B_LOC):
                        pA = psp.tile([128, 2, N_MM], F32, tag="pA",
                                      name=f"p1A_{t}_{b}_{eg}")
                        pB = psp.tile([128, 2, N_MM], F32, tag="pB",
                                      name=f"p1B_{t}_{b}_{eg}")
                        psums[(b, eg)] = (pA, pB)
                        lasts = []
                        for tgt, p in ((0, pA), (1, pB)):
                            for off in range(9):
                                dy, dx = off // 3, off % 3
                                for half in range(2):
                                    r0 = eg * EG_ROWS + half * CH_ROWS
                                    rhs = A1[0:128, b, r0 + dy:r0 + dy + CH_ROWS, dx:dx + W]
                                    mm = nc.tensor.matmul(p[:, half], lhsT=w1t[:, tgt, off],
                                                          rhs=rhs, start=(off == 0),
                                                          stop=(off == 8))
                                    if off == 8:
                                        lasts.append(mm.ins)
                                    if half == 0 and dy == 0:
                                        for hw_d, tt in ((h0w, t), (h1w, t - 1)):
                                            if (tt, eg - 1) in hw_d:
                                                _add_dep_helper(mm.ins, hw_d[(tt, eg - 1)],
                                                                reason="seam RAW dn")
                                    if half == 1 and dy == 2:
                                        for hw_d, tt in ((h0w, t), (h1w, t - 1)):
                                            if (tt, eg + 1) in hw_d:
                                                _add_dep_helper(mm.ins, hw_d[(tt, eg + 1)],
                                                                reason="seam RAW up")
                        mm1_last[(t, b, eg)] = lasts
                return psums

            def cell_ew(cell, t, psums):
                bcol = 2 * cell
                clo, chi = (0, 64) if cell == 0 else (64, 128)
                flo, fhi = (0, 64) if cell == 0 else (64, 128)
                ilo, ihi = (64, 128) if cell == 0 else (0, 64)
                hw_d = h0w if cell == 0 else h1w
                for eg in range(N_EG):
                    seg = slice(eg * EG_ROWS, (eg + 1) * EG_ROWS)
                    sA = work.tile([128, B_LOC, 2 * N_MM], F32, tag="sA",
                                   name=f"sA_{t}_{cell}_{eg}")
                    sB = work.tile([128, B_LOC, 2 * N_MM], F32, tag="sB",
                                   name=f"sB_{t}_{cell}_{eg}")
                    M = work.tile([128, B_LOC, 2 * N_MM], F32, tag="M",
                                  name=f"M_{t}_{cell}_{eg}")
                    for b in range(B_LOC):
                        pA, pB = psums[(b, eg)]
                        nc.scalar.activation(out=sA[:, b],
                                             in_=pA.rearrange("p a b -> p (a b)"),
                                             func=SIG, bias=b_sb[:, bcol:bcol + 1])
                        nc.scalar.activation(out=sB[:, b],
                                             in_=pB.rearrange("p a b -> p (a b)"),
                                             func=SIG, bias=b_sb[:, bcol + 1:bcol + 2])
                    cseg = ct01[clo:chi, :, seg].rearrange("p a b c -> p a (b c)")
                    nc.scalar.activation(out=M[ilo:ihi], in_=sB[64:128], func=COPY,
                                         bias=-1.0, scale=2.0)
                    nc.vector.tensor_mul(sB[64:128], sA[ilo:ihi], M[ilo:ihi])
                    nc.vector.tensor_mul(M[64:128], sA[flo:fhi], cseg)
                    nc.vector.tensor_add(cseg, sB[64:128], M[64:128])
                    nc.scalar.activation(out=M[0:64], in_=cseg, func=TANH)
                    hdst = A1[64 * cell:64 * cell + 64, :,
                              1 + eg * EG_ROWS:1 + (eg + 1) * EG_ROWS, 1:1 + W]
                    hw = nc.vector.tensor_mul(hdst, sB[0:64], M[0:64])
                    hw_d[(t, eg)] = hw.ins
                    if cell == 0:
                        for dg in (-1, 1):
                            for b in range(B_LOC):
                                if (t - 1, b, eg + dg) in mm1_last:
                                    for mm in mm1_last[(t - 1, b, eg + dg)]:
                                        _add_dep_helper(hw.ins, mm, reason="h0 seam WAR")
                    else:
                        for dg in (-1, 1):
                            for b in range(B_LOC):
                                if (t, b, eg + dg) in mm1_last:
                                    for mm in mm1_last[(t, b, eg + dg)]:
                                        _add_dep_helper(hw.ins, mm, reason="h1 seam WAR")

            for t in range(T):
                ps0 = cell0_convs(t)
                cell_ew(0, t, ps0)
                if t + 1 < T:
                    # refresh the shifted h0 views for the next step; flat
                    # k-element shifts are contiguous DMAs whose wrap elements
                    # land on padding or never-read columns
                    A0f = A0.rearrange("p a b c -> p (a b c)")
                    A0cf = A0c.rearrange("p a b c -> p (a b c)")
                    A0df = A0d.rearrange("p a b c -> p (a b c)")
                    A1f = A1.rearrange("p a b c -> p (a b c)")
                    n = B_LOC * HP * WP
                    nc.sync.dma_start(out=A0f[0:64], in_=A1f[0:64])
                    nc.sync.dma_start(out=A0f[64:128, 0:n - 1], in_=A1f[0:64, 1:n])
                    nc.sync.dma_start(out=A0cf[0:64, 0:n - 2], in_=A1f[0:64, 2:n])
                    nc.sync.dma_start(out=A0cf[64:128, 0:n - 68], in_=A1f[0:64, 68:n])
                    nc.sync.dma_start(out=A0df[64:128, 0:n - 2], in_=A1f[0:64, 2:n])
                    nc.sync.dma_start(out=A0d[0:9], in_=x9_d[t + 1])
                ps1 = cell1_convs(t)
                cell_ew(1, t, ps1)

            for b in range(B_LOC):
                for ch in range(H // CH_ROWS):
                    p_h = psp.tile([1, N_MM], F32, tag="pA", name=f"ph_{b}_{ch}")
                    rhs = A1[64:128, b, 1 + ch * CH_ROWS:1 + (ch + 1) * CH_ROWS, 1:1 + W]
                    mh = nc.tensor.matmul(p_h, lhsT=whT[64:128], rhs=rhs,
                                          start=True, stop=True)
                    if (T - 1, ch // 2) in h1w:
                        _add_dep_helper(mh.ins, h1w[(T - 1, ch // 2)], reason="head RAW")
                    h_out = work.tile([1, N_MM], F32, tag="ho", name=f"ho_{b}_{ch}")
                    nc.scalar.activation(out=h_out, in_=p_h, func=RELU,
                                         bias=bh_sb[0:1, 0:1])
                    nc.sync.dma_start(out=y_d[b:b + 1, ch * N_MM:(ch + 1) * N_MM],
                                      in_=h_out)

    nc.compile()
    return nc


def _prep_inputs(x, w0, b0, w1, b1, wh, bh):
    x = np.asarray(x, np.float32)
    B, T = x.shape[0], x.shape[1]
    bl = B // N_CORES
    BF = ml_dtypes.bfloat16

    def gate_blocks(w, b, K_, a_order):
        w = np.asarray(w, np.float32).reshape(4, 64, K_, 3, 3)
        b = np.asarray(b, np.float32).reshape(4, 64)
        wi, wf, wo, wg = w
        bi, bf, bo, bg = b
        if a_order == "fi":
            wA = np.concatenate([wf, wi], axis=0)
            bA = np.concatenate([bf, bi])
        else:
            wA = np.concatenate([wi, wf], axis=0)
            bA = np.concatenate([bi, bf])
        wB = np.concatenate([wo, 2.0 * wg], axis=0)
        bB = np.concatenate([bo, 2.0 * bg])
        return wA, wB, bA, bB

    # cell0: input channels [x(1); h0(64)]; A = [f; i]
    w0A, w0B, b0A, b0B = gate_blocks(w0, b0, 65, "fi")
    w0p = np.zeros((128, 2, 3, 128), np.float32)
    w0c = np.zeros((128, 2, 128), np.float32)
    w0d = np.zeros((128, 2, 128), np.float32)
    for tgt, wt in ((0, w0A), (1, w0B)):
        wh_ = wt[:, 1:65]   # [128col, 64ch, 3, 3]
        for dy in range(3):
            w0p[0:64, tgt, dy, :] = wh_[:, :, dy, 0].T
            w0p[64:128, tgt, dy, :] = wh_[:, :, dy, 1].T
        w0c[0:64, tgt, :] = wh_[:, :, 0, 2].T     # tap (0,2)
        w0c[64:128, tgt, :] = wh_[:, :, 1, 2].T   # tap (1,2)
        w0d[64:128, tgt, :] = wh_[:, :, 2, 2].T   # tap (2,2)
        for dy in range(3):
            for dx in range(3):
                w0d[3 * dy + dx, tgt, :] = wt[:, 0, dy, dx]   # x taps

    # cell1: input channels [h0(64); h1(64)]; A = [i; f]
    w1A, w1B, b1A, b1B = gate_blocks(w1, b1, 128, "if")
    w1t = np.zeros((128, 2, 9, 128), np.float32)
    for tgt, wt in ((0, w1A), (1, w1B)):
        w1t[:, tgt, :, :] = np.transpose(wt.reshape(128, 128, 9), (1, 2, 0))

    bt = np.stack([b0A, b0B, b1A, b1B], axis=1).astype(np.float32)

    wht = np.zeros((128, 1), np.float32)
    wht[64:128, 0] = np.asarray(wh, np.float32).reshape(64)
    bht = np.array([[float(np.asarray(bh).reshape(-1)[0])]], np.float32)

    # x9 im2col for the A0d matmul (read at rows r0+2, cols 0:W):
    # x9[t, 3dy+dx, b, row, c] = x_pad[b, t, row-2+dy, c+dx]
    xpad = np.zeros((B, T, HP, WP), np.float32)
    xpad[:, :, 1:1 + H, 1:1 + W] = x[:, :, 0]
    x9_all = np.zeros((B, T, 9, HP, WP), np.float32)
    for dy in range(3):
        for dx in range(3):
            rd0 = max(0, 2 - dy)            # dst row range whose src is valid
            rd1 = min(HP, HP + 2 - dy)
            x9_all[:, :, 3 * dy + dx, rd0:rd1, 0:W] = \
                xpad[:, :, rd0 - 2 + dy:rd1 - 2 + dy, dx:dx + W]

    in_maps = []
    for c in range(N_CORES):
        x9 = np.ascontiguousarray(
            x9_all[c * bl:(c + 1) * bl].transpose(1, 2, 0, 3, 4)).astype(BF)
        in_maps.append({"x9": x9, "w0p": w0p.astype(BF), "w0c": w0c.astype(BF),
                        "w0d": w0d.astype(BF), "w1t": w1t.astype(BF), "bt": bt,
                        "wht": wht.astype(BF), "bht": bht})
    return in_maps


_NC_CACHE = {}


def kernel(x, w0, b0, w1, b1, wh, bh):
    x = np.asarray(x)
    B, T = x.shape[0], x.shape[1]
    if T not in _NC_CACHE:
        _NC_CACHE[T] = _build(T=T)
    nc = _NC_CACHE[T]
    in_maps = _prep_inputs(x, w0, b0, w1, b1, wh, bh)
    res = run_bass_kernel_spmd(nc, in_maps, core_ids=list(range(N_CORES)))
    bl = B // N_CORES
    out = np.zeros((B, 1, H, W), np.float32)
    for c, r in enumerate(res.results):
        out[c * bl:(c + 1) * bl, 0] = r["y"].reshape(bl, H, W)
    return out


# revision 7
# speedup vs baseline: 1.1732x; 1.0067x over previous
"""ConvLSTM (2-layer, HID=64, 64x64, T=16, B=16) Trainium2 Bass kernel.

Sharding: data-parallel over batch B=16 -> 2 per NeuronCore across 8 cores;
weights/biases replicated; the sequential T-loop runs locally per core.
Matmul operands are bf16 (PSUM accumulation and the state update stay fp32);
bf16 streams at full PE rate with fast weight loads and halves SBUF, which
is what makes the 5-matmul cell0 decomposition below fit.

cell0 (65->256, 3x3) runs FIVE full-width K=128 matmuls per 128-out x
512-pos gate tile (vs 9 half-empty K=65 ones naively):
  - 3 paired-tap matmuls vs A0  = [h0 ; h0<<1col]   (taps (dy,0)+(dy,1))
  - 1 paired-tap matmul  vs A0c = [h0<<2col ; h0<<2col,+1row]  ((0,2)+(1,2))
  - 1 fused matmul       vs A0d = [x im2col 9 planes ; 0 ; h0<<2col]
    covering h tap (2,2) plus ALL NINE x taps on partitions 0:9
At t=0 all h taps vanish (h0=0): one matmul per gate tile.
cell1 (128->256, 3x3) is 9 K=128 shifted-window matmuls per gate tile
(PE-optimal). Head: K=64 1x1 matmuls + fused bias-relu ACT.

The h0 shift copies are flat 1/2/68-element-shifted contiguous DMAs from
A1[0:64] each step; row-wrap elements land on padding or unread columns.

Elementwise: gates packed [f;i] (cell0) / [i;f] (cell1) and [o;2g]; every
sigmoid is a 128-partition ACT; tanh(g) = 2*sigmoid(2g)-1 with the 2x
folded into the g-weights; DVE ops pair the 2 local batches ([64,2048]).
All two-input engine ops keep equal partition bases (HW rule); placement
via output partition shifts. Seam row overlaps Tile's subtile tracker
misses get explicit dependency edges.

States in SBUF: A0/A0c/A0d/A1 [128, 2, 66, 66] bf16,
ct01 [128, 2, 64, 64] f32 (c0 at parts 0:64, c1 at parts 64:128).
"""
import numpy as np
import ml_dtypes
import concourse.tile as tile
from concourse import mybir, bacc
from concourse.bass import _add_dep_helper
from concourse.bass_utils import run_bass_kernel_spmd

F32 = mybir.dt.float32
BF16 = mybir.dt.bfloat16
SIG = mybir.ActivationFunctionType.Sigmoid
TANH = mybir.ActivationFunctionType.Tanh
COPY = mybir.ActivationFunctionType.Copy
RELU = mybir.ActivationFunctionType.Relu

N_CORES = 8
B_LOC = 2
H = W = 64
HP = WP = 66
EG_ROWS = 16
CH_ROWS = 8
N_EG = H // EG_ROWS  # 4
N_MM = CH_ROWS * W  # 512


def _build(T=16):
    nc = bacc.Bacc("TRN2", target_bir_lowering=False, debug=False, num_devices=N_CORES)

    x9_d = nc.dram_tensor("x9", [T, 9, B_LOC, HP, WP], BF16,
                          kind="ExternalInput").ap()
    w0p_d = nc.dram_tensor("w0p", [128, 2, 3, 128], BF16, kind="ExternalInput").ap()
    w0c_d = nc.dram_tensor("w0c", [128, 2, 128], BF16, kind="ExternalInput").ap()
    w0d_d = nc.dram_tensor("w0d", [128, 2, 128], BF16, kind="ExternalInput").ap()
    w1_d = nc.dram_tensor("w1t", [128, 2, 9, 128], BF16, kind="ExternalInput").ap()
    b_d = nc.dram_tensor("bt", [128, 4], F32, kind="ExternalInput").ap()
    wh_d = nc.dram_tensor("wht", [128, 1], BF16, kind="ExternalInput").ap()
    bh_d = nc.dram_tensor("bht", [1, 1], F32, kind="ExternalInput").ap()
    y_d = nc.dram_tensor("y", [B_LOC, H * W], F32, kind="ExternalOutput").ap()

    with tile.TileContext(nc) as tc:
        with tc.tile_pool(name="state", bufs=1) as state, \
                tc.tile_pool(name="work", bufs=3) as work, \
                tc.tile_pool(name="psp", bufs=2, space="PSUM") as psp:
            A0 = state.tile([128, B_LOC, HP, WP], BF16)
            A0c = state.tile([128, B_LOC, HP, WP], BF16)
            A0d = state.tile([128, B_LOC, HP, WP], BF16)
            A1 = state.tile([128, B_LOC, HP, WP], BF16)
            ct01 = state.tile([128, B_LOC, H, W], F32)
            w0p = state.tile([128, 2, 3, 128], BF16)
            w0c = state.tile([128, 2, 128], BF16)
            w0d = state.tile([128, 2, 128], BF16)
            w1t = state.tile([128, 2, 9, 128], BF16)
            b_sb = state.tile([128, 4], F32)
            whT = state.tile([128, 1], BF16)
            bh_sb = state.tile([1, 1], F32)

            # t=0-critical first: w0d + A0d (its memset + the x im2col DMA)
            nc.sync.dma_start(out=w0d, in_=w0d_d)
            nc.vector.memset(A0d.bitcast(mybir.dt.uint16), 0)
            nc.sync.dma_start(out=A0d[0:9], in_=x9_d[0])
            nc.sync.dma_start(out=w0p, in_=w0p_d)
            nc.sync.dma_start(out=w0c, in_=w0c_d)
            nc.sync.dma_start(out=w1t, in_=w1_d)
            nc.sync.dma_start(out=b_sb, in_=b_d)
            nc.sync.dma_start(out=whT, in_=wh_d)
            nc.sync.dma_start(out=bh_sb, in_=bh_d)

            nc.vector.memset(A1.bitcast(mybir.dt.uint16), 0)
            nc.vector.memset(ct01, 0.0)
            nc.vector.memset(A0.bitcast(mybir.dt.uint16), 0)
            nc.vector.memset(A0c.bitcast(mybir.dt.uint16), 0)

            h0w = {}   # (t, eg) -> h0-write instruction (covers both b)
            h1w = {}   # (t, eg) -> h1-write instruction
            mm1_last = {}  # (t, b, eg) -> last matmuls of cell1 group

            def cell0_convs(t):
                psums = {}
                # eg-outer to match the ew phase's consumption order (the
                # ScalarE FIFO + 2-deep psum pool deadlock otherwise)
                for eg in range(N_EG):
                    for b in range(B_LOC):
                        pA = psp.tile([128, 2, N_MM], F32, tag="pA",
                                      name=f"p0A_{t}_{b}_{eg}")
                        pB = psp.tile([128, 2, N_MM], F32, tag="pB",
                                      name=f"p0B_{t}_{b}_{eg}")
                        psums[(b, eg)] = (pA, pB)
                        for tgt, p in ((0, pA), (1, pB)):
                            # t=0: h0 == 0, only the x-carrying matmul matters
                            for dy in (range(3) if t > 0 else ()):
                                for half in range(2):
                                    r0 = eg * EG_ROWS + half * CH_ROWS
                                    rhs = A0[0:128, b, r0 + dy:r0 + dy + CH_ROWS, 0:W]
                                    nc.tensor.matmul(p[:, half], lhsT=w0p[:, tgt, dy],
                                                     rhs=rhs, start=(dy == 0),
                                                     stop=False)
                            if t > 0:
                                for half in range(2):
                                    r0 = eg * EG_ROWS + half * CH_ROWS
                                    rhs = A0c[0:128, b, r0:r0 + CH_ROWS, 0:W]
                                    nc.tensor.matmul(p[:, half], lhsT=w0c[:, tgt],
                                                     rhs=rhs, start=False, stop=False)
                            for half in range(2):
                                r0 = eg * EG_ROWS + half * CH_ROWS
                                rhs = A0d[0:128, b, r0 + 2:r0 + 2 + CH_ROWS, 0:W]
                                nc.tensor.matmul(p[:, half], lhsT=w0d[:, tgt],
                                                 rhs=rhs, start=(t == 0), stop=True)
                return psums

            def cell1_convs(t):
                psums = {}
                for eg in range(N_EG):
                    for b in range(B_LOC):
                        pA = psp.tile([128, 2, N_MM], F32, tag="pA",
                                      name=f"p1A_{t}_{b}_{eg}")
                        pB = psp.tile([128, 2, N_MM], F32, tag="pB",
                                      name=f"p1B_{t}_{b}_{eg}")
                        psums[(b, eg)] = (pA, pB)
                        lasts = []
                        for tgt, p in ((0, pA), (1, pB)):
                            for half in range(2):
                                # issue the seam-row taps (dy=0 for half0,
                                # dy=2 for half1) LAST so the neighboring eg's
                                # h-write has maximal slack to land
                                offs = (3, 4, 5, 6, 7, 8, 0, 1, 2) if half == 0 \
                                    else tuple(range(9))
                                for k, off in enumerate(offs):
                                    dy, dx = off // 3, off % 3
                                    r0 = eg * EG_ROWS + half * CH_ROWS
                                    rhs = A1[0:128, b, r0 + dy:r0 + dy + CH_ROWS, dx:dx + W]
                                    mm = nc.tensor.matmul(p[:, half], lhsT=w1t[:, tgt, off],
                                                          rhs=rhs, start=(k == 0),
                                                          stop=(k == 8))
                                    if k == 8:
                                        lasts.append(mm.ins)
                                    if half == 0 and dy == 0:
                                        for hw_d, tt in ((h0w, t), (h1w, t - 1)):
                                            if (tt, eg - 1) in hw_d:
                                                _add_dep_helper(mm.ins, hw_d[(tt, eg - 1)],
                                                                reason="seam RAW dn")
                                    if half == 1 and dy == 2:
                                        for hw_d, tt in ((h0w, t), (h1w, t - 1)):
                                            if (tt, eg + 1) in hw_d:
                                                _add_dep_helper(mm.ins, hw_d[(tt, eg + 1)],
                                                                reason="seam RAW up")
                        mm1_last[(t, b, eg)] = lasts
                return psums

            def cell_ew(cell, t, psums):
                bcol = 2 * cell
                clo, chi = (0, 64) if cell == 0 else (64, 128)
                flo, fhi = (0, 64) if cell == 0 else (64, 128)
                ilo, ihi = (64, 128) if cell == 0 else (0, 64)
                hw_d = h0w if cell == 0 else h1w
                for eg in range(N_EG):
                    seg = slice(eg * EG_ROWS, (eg + 1) * EG_ROWS)
                    sA = work.tile([128, B_LOC, 2 * N_MM], F32, tag="sA",
                                   name=f"sA_{t}_{cell}_{eg}")
                    sB = work.tile([128, B_LOC, 2 * N_MM], F32, tag="sB",
                                   name=f"sB_{t}_{cell}_{eg}")
                    M = work.tile([128, B_LOC, 2 * N_MM], F32, tag="M",
                                  name=f"M_{t}_{cell}_{eg}")
                    for b in range(B_LOC):
                        pA, pB = psums[(b, eg)]
                        nc.scalar.activation(out=sA[:, b],
                                             in_=pA.rearrange("p a b -> p (a b)"),
                                             func=SIG, bias=b_sb[:, bcol:bcol + 1])
                        nc.scalar.activation(out=sB[:, b],
                                             in_=pB.rearrange("p a b -> p (a b)"),
                                             func=SIG, bias=b_sb[:, bcol + 1:bcol + 2])
                    cseg = ct01[clo:chi, :, seg].rearrange("p a b c -> p a (b c)")
                    nc.scalar.activation(out=M[ilo:ihi], in_=sB[64:128], func=COPY,
                                         bias=-1.0, scale=2.0)
                    if t == 0:
                        # c == 0: c' = sig(i)*tg directly; skipping m1/add
                        # shortens the ew chain right where TensorE is
                        # starved for the first h-writes
                        nc.vector.tensor_mul(cseg, sA[ilo:ihi], M[ilo:ihi])
                    else:
                        nc.vector.tensor_mul(sB[64:128], sA[ilo:ihi], M[ilo:ihi])
                        nc.vector.tensor_mul(M[64:128], sA[flo:fhi], cseg)
                        nc.vector.tensor_add(cseg, sB[64:128], M[64:128])
                    nc.scalar.activation(out=M[0:64], in_=cseg, func=TANH)
                    hdst = A1[64 * cell:64 * cell + 64, :,
                              1 + eg * EG_ROWS:1 + (eg + 1) * EG_ROWS, 1:1 + W]
                    hw = nc.vector.tensor_mul(hdst, sB[0:64], M[0:64])
                    hw_d[(t, eg)] = hw.ins
                    if cell == 0:
                        for dg in (-1, 1):
                            for b in range(B_LOC):
                                if (t - 1, b, eg + dg) in mm1_last:
                                    for mm in mm1_last[(t - 1, b, eg + dg)]:
                                        _add_dep_helper(hw.ins, mm, reason="h0 seam WAR")
                    else:
                        for dg in (-1, 1):
                            for b in range(B_LOC):
                                if (t, b, eg + dg) in mm1_last:
                                    for mm in mm1_last[(t, b, eg + dg)]:
                                        _add_dep_helper(hw.ins, mm, reason="h1 seam WAR")

            for t in range(T):
                ps0 = cell0_convs(t)
                cell_ew(0, t, ps0)
                if t + 1 < T:
                    # refresh the shifted h0 views for the next step; flat
                    # k-element shifts are contiguous DMAs whose wrap elements
                    # land on padding or never-read columns
                    A0f = A0.rearrange("p a b c -> p (a b c)")
                    A0cf = A0c.rearrange("p a b c -> p (a b c)")
                    A0df = A0d.rearrange("p a b c -> p (a b c)")
                    A1f = A1.rearrange("p a b c -> p (a b c)")
                    n = B_LOC * HP * WP
                    nc.sync.dma_start(out=A0f[0:64], in_=A1f[0:64])
                    nc.sync.dma_start(out=A0f[64:128, 0:n - 1], in_=A1f[0:64, 1:n])
                    nc.sync.dma_start(out=A0cf[0:64, 0:n - 2], in_=A1f[0:64, 2:n])
                    nc.sync.dma_start(out=A0cf[64:128, 0:n - 68], in_=A1f[0:64, 68:n])
                    nc.sync.dma_start(out=A0df[64:128, 0:n - 2], in_=A1f[0:64, 2:n])
                    nc.sync.dma_start(out=A0d[0:9], in_=x9_d[t + 1])
                ps1 = cell1_convs(t)
                cell_ew(1, t, ps1)

            for b in range(B_LOC):
                for ch in range(H // CH_ROWS):
                    p_h = psp.tile([1, N_MM], F32, tag="pA", name=f"ph_{b}_{ch}")
                    rhs = A1[64:128, b, 1 + ch * CH_ROWS:1 + (ch + 1) * CH_ROWS, 1:1 + W]
                    mh = nc.tensor.matmul(p_h, lhsT=whT[64:128], rhs=rhs,
                                          start=True, stop=True)
                    if (T - 1, ch // 2) in h1w:
                        _add_dep_helper(mh.ins, h1w[(T - 1, ch // 2)], reason="head RAW")
                    h_out = work.tile([1, N_MM], F32, tag="ho", name=f"ho_{b}_{ch}")
                    nc.scalar.activation(out=h_out, in_=p_h, func=RELU,
                                         bias=bh_sb[0:1, 0:1])
                    nc.sync.dma_start(out=y_d[b:b + 1, ch * N_MM:(ch + 1) * N_MM],
                                      in_=h_out)

    nc.compile()
    return nc


def _prep_inputs(x, w0, b0, w1, b1, wh, bh):
    x = np.asarray(x, np.float32)
    B, T = x.shape[0], x.shape[1]
    bl = B // N_CORES
    BF = ml_dtypes.bfloat16

    def gate_blocks(w, b, K_, a_order):
        w = np.asarray(w, np.float32).reshape(4, 64, K_, 3, 3)
        b = np.asarray(b, np.float32).reshape(4, 64)
        wi, wf, wo, wg = w
        bi, bf, bo, bg = b
        if a_order == "fi":
            wA = np.concatenate([wf, wi], axis=0)
            bA = np.concatenate([bf, bi])
        else:
            wA = np.concatenate([wi, wf], axis=0)
            bA = np.concatenate([bi, bf])
        wB = np.concatenate([wo, 2.0 * wg], axis=0)
        bB = np.concatenate([bo, 2.0 * bg])
        return wA, wB, bA, bB

    # cell0: input channels [x(1); h0(64)]; A = [f; i]
    w0A, w0B, b0A, b0B = gate_blocks(w0, b0, 65, "fi")
    w0p = np.zeros((128, 2, 3, 128), np.float32)
    w0c = np.zeros((128, 2, 128), np.float32)
    w0d = np.zeros((128, 2, 128), np.float32)
    for tgt, wt in ((0, w0A), (1, w0B)):
        wh_ = wt[:, 1:65]   # [128col, 64ch, 3, 3]
        for dy in range(3):
            w0p[0:64, tgt, dy, :] = wh_[:, :, dy, 0].T
            w0p[64:128, tgt, dy, :] = wh_[:, :, dy, 1].T
        w0c[0:64, tgt, :] = wh_[:, :, 0, 2].T     # tap (0,2)
        w0c[64:128, tgt, :] = wh_[:, :, 1, 2].T   # tap (1,2)
        w0d[64:128, tgt, :] = wh_[:, :, 2, 2].T   # tap (2,2)
        for dy in range(3):
            for dx in range(3):
                w0d[3 * dy + dx, tgt, :] = wt[:, 0, dy, dx]   # x taps

    # cell1: input channels [h0(64); h1(64)]; A = [i; f]
    w1A, w1B, b1A, b1B = gate_blocks(w1, b1, 128, "if")
    w1t = np.zeros((128, 2, 9, 128), np.float32)
    for tgt, wt in ((0, w1A), (1, w1B)):
        w1t[:, tgt, :, :] = np.transpose(wt.reshape(128, 128, 9), (1, 2, 0))

    bt = np.stack([b0A, b0B, b1A, b1B], axis=1).astype(np.float32)

    wht = np.zeros((128, 1), np.float32)
    wht[64:128, 0] = np.asarray(wh, np.float32).reshape(64)
    bht = np.array([[float(np.asarray(bh).reshape(-1)[0])]], np.float32)

    # x9 im2col for the A0d matmul (read at rows r0+2, cols 0:W):
    # x9[t, 3dy+dx, b, row, c] = x_pad[b, t, row-2+dy, c+dx]
    xpad = np.zeros((B, T, HP, WP), np.float32)
    xpad[:, :, 1:1 + H, 1:1 + W] = x[:, :, 0]
    x9_all = np.zeros((B, T, 9, HP, WP), np.float32)
    for dy in range(3):
        for dx in range(3):
            rd0 = max(0, 2 - dy)            # dst row range whose src is valid
            rd1 = min(HP, HP + 2 - dy)
            x9_all[:, :, 3 * dy + dx, rd0:rd1, 0:W] = \
                xpad[:, :, rd0 - 2 + dy:rd1 - 2 + dy, dx:dx + W]

    in_maps = []
    for c in range(N_CORES):
        x9 = np.ascontiguousarray(
            x9_all[c * bl:(c + 1) * bl].transpose(1, 2, 0, 3, 4)).astype(BF)
        in_maps.append({"x9": x9, "w0p": w0p.astype(BF), "w0c": w0c.astype(BF),
                        "w0d": w0d.astype(BF), "w1t": w1t.astype(BF), "bt": bt,
                        "wht": wht.astype(BF), "bht": bht})
    return in_maps


_NC_CACHE = {}


def kernel(x, w0, b0, w1, b1, wh, bh):
    x = np.asarray(x)
    B, T = x.shape[0], x.shape[1]
    if T not in _NC_CACHE:
        _NC_CACHE[T] = _build(T=T)
    nc = _NC_CACHE[T]
    in_maps = _prep_inputs(x, w0, b0, w1, b1, wh, bh)
    res = run_bass_kernel_spmd(nc, in_maps, core_ids=list(range(N_CORES)))
    bl = B // N_CORES
    out = np.zeros((B, 1, H, W), np.float32)
    for c, r in enumerate(res.results):
        out[c * bl:(c + 1) * bl, 0] = r["y"].reshape(bl, H, W)
    return out


# revision 8
# speedup vs baseline: 1.1775x; 1.0036x over previous
"""ConvLSTM (2-layer, HID=64, 64x64, T=16, B=16) Trainium2 Bass kernel.

Sharding: data-parallel over batch B=16 -> 2 per NeuronCore across 8 cores;
weights/biases replicated; the sequential T-loop runs locally per core.
Matmul operands are bf16 (PSUM accumulation and the state update stay fp32);
bf16 streams at full PE rate with fast weight loads and halves SBUF, which
is what makes the 5-matmul cell0 decomposition below fit.

cell0 (65->256, 3x3) runs FIVE full-width K=128 matmuls per 128-out x
512-pos gate tile (vs 9 half-empty K=65 ones naively):
  - 3 paired-tap matmuls vs A0  = [h0 ; h0<<1col]   (taps (dy,0)+(dy,1))
  - 1 paired-tap matmul  vs A0c = [h0<<2col ; h0<<2col,+1row]  ((0,2)+(1,2))
  - 1 fused matmul       vs A0d = [x im2col 9 planes ; 0 ; h0<<2col]
    covering h tap (2,2) plus ALL NINE x taps on partitions 0:9
At t=0 all h taps vanish (h0=0): one matmul per gate tile.
cell1 (128->256, 3x3) is 9 K=128 shifted-window matmuls per gate tile
(PE-optimal). Head: K=64 1x1 matmuls + fused bias-relu ACT.

The h0 shift copies are flat 1/2/68-element-shifted contiguous DMAs from
A1[0:64] each step; row-wrap elements land on padding or unread columns.

Elementwise: gates packed [f;i] (cell0) / [i;f] (cell1) and [o;2g]; every
sigmoid is a 128-partition ACT; tanh(g) = 2*sigmoid(2g)-1 with the 2x
folded into the g-weights; DVE ops pair the 2 local batches ([64,2048]).
All two-input engine ops keep equal partition bases (HW rule); placement
via output partition shifts. Seam row overlaps Tile's subtile tracker
misses get explicit dependency edges.

States in SBUF: A0/A0c/A0d/A1 [128, 2, 66, 66] bf16,
ct01 [128, 2, 64, 64] f32 (c0 at parts 0:64, c1 at parts 64:128).
"""
import numpy as np
import ml_dtypes
import concourse.tile as tile
from concourse import mybir, bacc
from concourse.bass import _add_dep_helper
from concourse.bass_utils import run_bass_kernel_spmd

F32 = mybir.dt.float32
BF16 = mybir.dt.bfloat16
SIG = mybir.ActivationFunctionType.Sigmoid
TANH = mybir.ActivationFunctionType.Tanh
COPY = mybir.ActivationFunctionType.Copy
RELU = mybir.ActivationFunctionType.Relu

N_CORES = 8
B_LOC = 2
H = W = 64
HP = WP = 66
EG_ROWS = 16
CH_ROWS = 8
N_EG = H // EG_ROWS  # 4
N_MM = CH_ROWS * W  # 512


def _build(T=16):
    nc = bacc.Bacc("TRN2", target_bir_lowering=False, debug=False, num_devices=N_CORES)

    x9_d = nc.dram_tensor("x9", [T, 9, B_LOC, HP, WP], BF16,
                          kind="ExternalInput").ap()
    w0p_d = nc.dram_tensor("w0p", [128, 2, 3, 128], BF16, kind="ExternalInput").ap()
    w0c_d = nc.dram_tensor("w0c", [128, 2, 128], BF16, kind="ExternalInput").ap()
    w0d_d = nc.dram_tensor("w0d", [128, 2, 128], BF16, kind="ExternalInput").ap()
    w1_d = nc.dram_tensor("w1t", [128, 2, 9, 128], BF16, kind="ExternalInput").ap()
    b_d = nc.dram_tensor("bt", [128, 4], F32, kind="ExternalInput").ap()
    wh_d = nc.dram_tensor("wht", [128, 1], BF16, kind="ExternalInput").ap()
    bh_d = nc.dram_tensor("bht", [1, 1], F32, kind="ExternalInput").ap()
    y_d = nc.dram_tensor("y", [B_LOC, H * W], F32, kind="ExternalOutput").ap()

    with tile.TileContext(nc) as tc:
        with tc.tile_pool(name="state", bufs=1) as state, \
                tc.tile_pool(name="work", bufs=3) as work, \
                tc.tile_pool(name="psp", bufs=2, space="PSUM") as psp:
            A0 = state.tile([128, B_LOC, HP, WP], BF16)
            A0c = state.tile([128, B_LOC, HP, WP], BF16)
            A0d = state.tile([128, B_LOC, HP, WP], BF16)
            A1 = state.tile([128, B_LOC, HP, WP], BF16)
            ct01 = state.tile([128, B_LOC, H, W], F32)
            w0p = state.tile([128, 2, 3, 128], BF16)
            w0c = state.tile([128, 2, 128], BF16)
            w0d = state.tile([128, 2, 128], BF16)
            w1t = state.tile([128, 2, 9, 128], BF16)
            b_sb = state.tile([128, 4], F32)
            whT = state.tile([128, 1], BF16)
            bh_sb = state.tile([1, 1], F32)

            # t=0-critical first: w0d + A0d (its memset + the x im2col DMA)
            nc.sync.dma_start(out=w0d, in_=w0d_d)
            nc.vector.memset(A0d.bitcast(mybir.dt.uint16), 0)
            nc.sync.dma_start(out=A0d[0:9], in_=x9_d[0])
            nc.sync.dma_start(out=w0p, in_=w0p_d)
            nc.sync.dma_start(out=w0c, in_=w0c_d)
            nc.sync.dma_start(out=w1t, in_=w1_d)
            nc.sync.dma_start(out=b_sb, in_=b_d)
            nc.sync.dma_start(out=whT, in_=wh_d)
            nc.sync.dma_start(out=bh_sb, in_=bh_d)

            nc.vector.memset(A1.bitcast(mybir.dt.uint16), 0)
            nc.vector.memset(ct01, 0.0)
            nc.vector.memset(A0.bitcast(mybir.dt.uint16), 0)
            nc.vector.memset(A0c.bitcast(mybir.dt.uint16), 0)

            h0w = {}   # (t, eg) -> h0-write instruction (covers both b)
            h1w = {}   # (t, eg) -> h1-write instruction
            mm1_last = {}  # (t, b, eg) -> last matmuls of cell1 group

            def cell0_convs(t):
                psums = {}
                # eg-outer to match the ew phase's consumption order (the
                # ScalarE FIFO + 2-deep psum pool deadlock otherwise)
                for eg in range(N_EG):
                    for b in range(B_LOC):
                        pA = psp.tile([128, 2, N_MM], F32, tag="pA",
                                      name=f"p0A_{t}_{b}_{eg}")
                        pB = psp.tile([128, 2, N_MM], F32, tag="pB",
                                      name=f"p0B_{t}_{b}_{eg}")
                        psums[(b, eg)] = (pA, pB)
                        for tgt, p in ((0, pA), (1, pB)):
                            # t=0: h0 == 0, only the x-carrying matmul matters
                            for dy in (range(3) if t > 0 else ()):
                                for half in range(2):
                                    r0 = eg * EG_ROWS + half * CH_ROWS
                                    rhs = A0[0:128, b, r0 + dy:r0 + dy + CH_ROWS, 0:W]
                                    nc.tensor.matmul(p[:, half], lhsT=w0p[:, tgt, dy],
                                                     rhs=rhs, start=(dy == 0),
                                                     stop=False)
                            if t > 0:
                                for half in range(2):
                                    r0 = eg * EG_ROWS + half * CH_ROWS
                                    rhs = A0c[0:128, b, r0:r0 + CH_ROWS, 0:W]
                                    nc.tensor.matmul(p[:, half], lhsT=w0c[:, tgt],
                                                     rhs=rhs, start=False, stop=False)
                            for half in range(2):
                                r0 = eg * EG_ROWS + half * CH_ROWS
                                rhs = A0d[0:128, b, r0 + 2:r0 + 2 + CH_ROWS, 0:W]
                                nc.tensor.matmul(p[:, half], lhsT=w0d[:, tgt],
                                                 rhs=rhs, start=(t == 0), stop=True)
                return psums

            def cell1_convs(t):
                psums = {}
                for eg in range(N_EG):
                    for b in range(B_LOC):
                        pA = psp.tile([128, 2, N_MM], F32, tag="pA",
                                      name=f"p1A_{t}_{b}_{eg}")
                        pB = psp.tile([128, 2, N_MM], F32, tag="pB",
                                      name=f"p1B_{t}_{b}_{eg}")
                        psums[(b, eg)] = (pA, pB)
                        lasts = []
                        for tgt, p in ((0, pA), (1, pB)):
                            for half in range(2):
                                # issue the seam-row taps (dy=0 for half0,
                                # dy=2 for half1) LAST so the neighboring eg's
                                # h-write has maximal slack to land
                                offs = (3, 4, 5, 6, 7, 8, 0, 1, 2) if half == 0 \
                                    else tuple(range(9))
                                for k, off in enumerate(offs):
                                    dy, dx = off // 3, off % 3
                                    r0 = eg * EG_ROWS + half * CH_ROWS
                                    rhs = A1[0:128, b, r0 + dy:r0 + dy + CH_ROWS, dx:dx + W]
                                    mm = nc.tensor.matmul(p[:, half], lhsT=w1t[:, tgt, off],
                                                          rhs=rhs, start=(k == 0),
                                                          stop=(k == 8))
                                    if k == 8:
                                        lasts.append(mm.ins)
                                    if half == 0 and dy == 0:
                                        for hw_d, tt in ((h0w, t), (h1w, t - 1)):
                                            if (tt, eg - 1) in hw_d:
                                                _add_dep_helper(mm.ins, hw_d[(tt, eg - 1)],
                                                                reason="seam RAW dn")
                                    if half == 1 and dy == 2:
                                        for hw_d, tt in ((h0w, t), (h1w, t - 1)):
                                            if (tt, eg + 1) in hw_d:
                                                _add_dep_helper(mm.ins, hw_d[(tt, eg + 1)],
                                                                reason="seam RAW up")
                        mm1_last[(t, b, eg)] = lasts
                return psums

            def cell_ew(cell, t, psums):
                bcol = 2 * cell
                clo, chi = (0, 64) if cell == 0 else (64, 128)
                flo, fhi = (0, 64) if cell == 0 else (64, 128)
                ilo, ihi = (64, 128) if cell == 0 else (0, 64)
                hw_d = h0w if cell == 0 else h1w
                for eg in range(N_EG):
                    seg = slice(eg * EG_ROWS, (eg + 1) * EG_ROWS)
                    sA = work.tile([128, B_LOC, 2 * N_MM], F32, tag="sA",
                                   name=f"sA_{t}_{cell}_{eg}")
                    sB = work.tile([128, B_LOC, 2 * N_MM], F32, tag="sB",
                                   name=f"sB_{t}_{cell}_{eg}")
                    M = work.tile([128, B_LOC, 2 * N_MM], F32, tag="M",
                                  name=f"M_{t}_{cell}_{eg}")
                    if t == 0:
                        # c == 0: c' = sig(i)*tg directly (skip m1/add), and
                        # run per-b so each op is half-size — the first
                        # h-writes gate cell1(0)'s start and each cell1
                        # group only reads its own batch
                        for b in range(B_LOC):
                            pA, pB = psums[(b, eg)]
                            nc.scalar.activation(out=sA[:, b],
                                                 in_=pA.rearrange("p a b -> p (a b)"),
                                                 func=SIG, bias=b_sb[:, bcol:bcol + 1])
                            nc.scalar.activation(out=sB[:, b],
                                                 in_=pB.rearrange("p a b -> p (a b)"),
                                                 func=SIG, bias=b_sb[:, bcol + 1:bcol + 2])
                            csegb = ct01[clo:chi, b, seg].rearrange("p a b -> p (a b)")
                            nc.scalar.activation(out=M[ilo:ihi, b], in_=sB[64:128, b],
                                                 func=COPY, bias=-1.0, scale=2.0)
                            nc.vector.tensor_mul(csegb, sA[ilo:ihi, b], M[ilo:ihi, b])
                            nc.scalar.activation(out=M[0:64, b], in_=csegb, func=TANH)
                            hdst = A1[64 * cell:64 * cell + 64, b,
                                      1 + eg * EG_ROWS:1 + (eg + 1) * EG_ROWS, 1:1 + W]
                            hw = nc.vector.tensor_mul(hdst, sB[0:64, b], M[0:64, b])
                            hw_d[(t, eg)] = hw.ins  # keep last (b1): conservative
                            if cell == 1:
                                for dg in (-1, 1):
                                    for bb in range(B_LOC):
                                        if (t, bb, eg + dg) in mm1_last:
                                            for mm in mm1_last[(t, bb, eg + dg)]:
                                                _add_dep_helper(hw.ins, mm,
                                                                reason="h1 seam WAR t0")
                        continue
                    for b in range(B_LOC):
                        pA, pB = psums[(b, eg)]
                        nc.scalar.activation(out=sA[:, b],
                                             in_=pA.rearrange("p a b -> p (a b)"),
                                             func=SIG, bias=b_sb[:, bcol:bcol + 1])
                        nc.scalar.activation(out=sB[:, b],
                                             in_=pB.rearrange("p a b -> p (a b)"),
                                             func=SIG, bias=b_sb[:, bcol + 1:bcol + 2])
                    cseg = ct01[clo:chi, :, seg].rearrange("p a b c -> p a (b c)")
                    nc.scalar.activation(out=M[ilo:ihi], in_=sB[64:128], func=COPY,
                                         bias=-1.0, scale=2.0)
                    nc.vector.tensor_mul(sB[64:128], sA[ilo:ihi], M[ilo:ihi])
                    nc.vector.tensor_mul(M[64:128], sA[flo:fhi], cseg)
                    nc.vector.tensor_add(cseg, sB[64:128], M[64:128])
                    nc.scalar.activation(out=M[0:64], in_=cseg, func=TANH)
                    hdst = A1[64 * cell:64 * cell + 64, :,
                              1 + eg * EG_ROWS:1 + (eg + 1) * EG_ROWS, 1:1 + W]
                    hw = nc.vector.tensor_mul(hdst, sB[0:64], M[0:64])
                    hw_d[(t, eg)] = hw.ins
                    if cell == 0:
                        for dg in (-1, 1):
                            for b in range(B_LOC):
                                if (t - 1, b, eg + dg) in mm1_last:
                                    for mm in mm1_last[(t - 1, b, eg + dg)]:
                                        _add_dep_helper(hw.ins, mm, reason="h0 seam WAR")
                    else:
                        for dg in (-1, 1):
                            for b in range(B_LOC):
                                if (t, b, eg + dg) in mm1_last:
                                    for mm in mm1_last[(t, b, eg + dg)]:
                                        _add_dep_helper(hw.ins, mm, reason="h1 seam WAR")

            for t in range(T):
                ps0 = cell0_convs(t)
                cell_ew(0, t, ps0)
                if t + 1 < T:
                    # refresh the shifted h0 views for the next step; flat
                    # k-element shifts are contiguous DMAs whose wrap elements
                    # land on padding or never-read columns
                    A0f = A0.rearrange("p a b c -> p (a b c)")
                    A0cf = A0c.rearrange("p a b c -> p (a b c)")
                    A0df = A0d.rearrange("p a b c -> p (a b c)")
                    A1f = A1.rearrange("p a b c -> p (a b c)")
                    n = B_LOC * HP * WP
                    nc.sync.dma_start(out=A0f[0:64], in_=A1f[0:64])
                    nc.sync.dma_start(out=A0f[64:128, 0:n - 1], in_=A1f[0:64, 1:n])
                    nc.sync.dma_start(out=A0cf[0:64, 0:n - 2], in_=A1f[0:64, 2:n])
                    nc.sync.dma_start(out=A0cf[64:128, 0:n - 68], in_=A1f[0:64, 68:n])
                    nc.sync.dma_start(out=A0df[64:128, 0:n - 2], in_=A1f[0:64, 2:n])
                    nc.sync.dma_start(out=A0d[0:9], in_=x9_d[t + 1])
                ps1 = cell1_convs(t)
                cell_ew(1, t, ps1)

            for b in range(B_LOC):
                for ch in range(H // CH_ROWS):
                    p_h = psp.tile([1, N_MM], F32, tag="pA", name=f"ph_{b}_{ch}")
                    rhs = A1[64:128, b, 1 + ch * CH_ROWS:1 + (ch + 1) * CH_ROWS, 1:1 + W]
                    mh = nc.tensor.matmul(p_h, lhsT=whT[64:128], rhs=rhs,
                                          start=True, stop=True)
                    if (T - 1, ch // 2) in h1w:
                        _add_dep_helper(mh.ins, h1w[(T - 1, ch // 2)], reason="head RAW")
                    h_out = work.tile([1, N_MM], F32, tag="ho", name=f"ho_{b}_{ch}")
                    nc.scalar.activation(out=h_out, in_=p_h, func=RELU,
                                         bias=bh_sb[0:1, 0:1])
                    nc.sync.dma_start(out=y_d[b:b + 1, ch * N_MM:(ch + 1) * N_MM],
                                      in_=h_out)

    nc.compile()
    return nc


def _prep_inputs(x, w0, b0, w1, b1, wh, bh):
    x = np.asarray(x, np.float32)
    B, T = x.shape[0], x.shape[1]
    bl = B // N_CORES
    BF = ml_dtypes.bfloat16

    def gate_blocks(w, b, K_, a_order):
        w = np.asarray(w, np.float32).reshape(4, 64, K_, 3, 3)
        b = np.asarray(b, np.float32).reshape(4, 64)
        wi, wf, wo, wg = w
        bi, bf, bo, bg = b
        if a_order == "fi":
            wA = np.concatenate([wf, wi], axis=0)
            bA = np.concatenate([bf, bi])
        else:
            wA = np.concatenate([wi, wf], axis=0)
            bA = np.concatenate([bi, bf])
        wB = np.concatenate([wo, 2.0 * wg], axis=0)
        bB = np.concatenate([bo, 2.0 * bg])
        return wA, wB, bA, bB

    # cell0: input channels [x(1); h0(64)]; A = [f; i]
    w0A, w0B, b0A, b0B = gate_blocks(w0, b0, 65, "fi")
    w0p = np.zeros((128, 2, 3, 128), np.float32)
    w0c = np.zeros((128, 2, 128), np.float32)
    w0d = np.zeros((128, 2, 128), np.float32)
    for tgt, wt in ((0, w0A), (1, w0B)):
        wh_ = wt[:, 1:65]   # [128col, 64ch, 3, 3]
        for dy in range(3):
            w0p[0:64, tgt, dy, :] = wh_[:, :, dy, 0].T
            w0p[64:128, tgt, dy, :] = wh_[:, :, dy, 1].T
        w0c[0:64, tgt, :] = wh_[:, :, 0, 2].T     # tap (0,2)
        w0c[64:128, tgt, :] = wh_[:, :, 1, 2].T   # tap (1,2)
        w0d[64:128, tgt, :] = wh_[:, :, 2, 2].T   # tap (2,2)
        for dy in range(3):
            for dx in range(3):
                w0d[3 * dy + dx, tgt, :] = wt[:, 0, dy, dx]   # x taps

    # cell1: input channels [h0(64); h1(64)]; A = [i; f]
    w1A, w1B, b1A, b1B = gate_blocks(w1, b1, 128, "if")
    w1t = np.zeros((128, 2, 9, 128), np.float32)
    for tgt, wt in ((0, w1A), (1, w1B)):
        w1t[:, tgt, :, :] = np.transpose(wt.reshape(128, 128, 9), (1, 2, 0))

    bt = np.stack([b0A, b0B, b1A, b1B], axis=1).astype(np.float32)

    wht = np.zeros((128, 1), np.float32)
    wht[64:128, 0] = np.asarray(wh, np.float32).reshape(64)
    bht = np.array([[float(np.asarray(bh).reshape(-1)[0])]], np.float32)

    # x9 im2col for the A0d matmul (read at rows r0+2, cols 0:W):
    # x9[t, 3dy+dx, b, row, c] = x_pad[b, t, row-2+dy, c+dx]
    xpad = np.zeros((B, T, HP, WP), np.float32)
    xpad[:, :, 1:1 + H, 1:1 + W] = x[:, :, 0]
    x9_all = np.zeros((B, T, 9, HP, WP), np.float32)
    for dy in range(3):
        for dx in range(3):
            rd0 = max(0, 2 - dy)            # dst row range whose src is valid
            rd1 = min(HP, HP + 2 - dy)
            x9_all[:, :, 3 * dy + dx, rd0:rd1, 0:W] = \
                xpad[:, :, rd0 - 2 + dy:rd1 - 2 + dy, dx:dx + W]

    in_maps = []
    for c in range(N_CORES):
        x9 = np.ascontiguousarray(
            x9_all[c * bl:(c + 1) * bl].transpose(1, 2, 0, 3, 4)).astype(BF)
        in_maps.append({"x9": x9, "w0p": w0p.astype(BF), "w0c": w0c.astype(BF),
                        "w0d": w0d.astype(BF), "w1t": w1t.astype(BF), "bt": bt,
                        "wht": wht.astype(BF), "bht": bht})
    return in_maps


_NC_CACHE = {}


def kernel(x, w0, b0, w1, b1, wh, bh):
    x = np.asarray(x)
    B, T = x.shape[0], x.shape[1]
    if T not in _NC_CACHE:
        _NC_CACHE[T] = _build(T=T)
    nc = _NC_CACHE[T]
    in_maps = _prep_inputs(x, w0, b0, w1, b1, wh, bh)
    res = run_bass_kernel_spmd(nc, in_maps, core_ids=list(range(N_CORES)))
    bl = B // N_CORES
    out = np.zeros((B, 1, H, W), np.float32)
    for c, r in enumerate(res.results):
        out[c * bl:(c + 1) * bl, 0] = r["y"].reshape(bl, H, W)
    return out
